# revision 46
# baseline (speedup 1.0000x reference)
"""Multi-head attention (B=4, N=2048, DIM=512, H=8) on 8 TRN2 NeuronCores.

Sharding: core c handles (batch = c//2, query-half = c%2) -> 1024 queries of
one batch, all heads. Zero collectives: K/V are recomputed per core pair
(keys are permuted so each core's queries come first; softmax is
permutation-invariant over keys).

Device layout ("transposed flash"):
  - everything dim-major: X^T, Q^T, K^T in SBUF with the contraction dim on
    partitions; V in natural [n, d] layout with a fused ones-column so the
    PV matmul also produces the softmax denominators (row 64 of the PSUM
    accumulator).
  - scores S^T = K_h^T-stationary @ Q_h^T-moving -> [nk_tile, nq] PSUM,
    exp on ScalarE (scale fused) -> P^T bf16 in SBUF,
    O^T_aug += V_aug^T-stationary @ P^T-moving accumulated over nk tiles.
  - normalization: reciprocal of the sums row, broadcast across partitions
    via a DRAM round-trip DMA (step-0 access pattern), multiply on VectorE.
  - output projection Wout-stationary gives Y^T [512, 1024]; host transposes.
"""

import os

import numpy as np
import ml_dtypes

B, N, DIM = 4, 2048, 512
H, D = 8, 64
NQ = 1024            # queries per core
NCORES = 8
SCALE = DIM ** -0.5  # reference scales by full dim, not head dim

BF16 = ml_dtypes.bfloat16

_CACHE = {}

LAST_EXEC_TIME_NS = None


def _build():
    import concourse.bass as bass
    import concourse.mybir as mybir
    import concourse.tile as tile
    from concourse import bacc

    f32 = mybir.dt.float32
    bf16 = mybir.dt.bfloat16
    Exp = mybir.ActivationFunctionType.Exp
    ts = bass.ts

    nc = bacc.Bacc("TRN2", target_bir_lowering=False, debug=False,
                   num_devices=NCORES)

    xt = nc.dram_tensor("xt", [DIM, N], bf16, kind="ExternalInput")
    wqkv = nc.dram_tensor("wqkv", [DIM, 3 * DIM], bf16, kind="ExternalInput")
    wout = nc.dram_tensor("wout", [DIM, DIM], bf16, kind="ExternalInput")
    bout = nc.dram_tensor("bout", [128, 4], f32, kind="ExternalInput")
    yt = nc.dram_tensor("out", [DIM, NQ], f32, kind="ExternalOutput")

    with tile.TileContext(nc) as tc:
        with (
            tc.tile_pool(name="persist", bufs=1) as persist,
            tc.tile_pool(name="ptiles", bufs=8) as ptiles,
            tc.tile_pool(name="norm", bufs=2) as norm,
            tc.tile_pool(name="ysb", bufs=2) as ysb,
            tc.tile_pool(name="psum_s", bufs=2, space="PSUM") as psum_s,
            tc.tile_pool(name="psum_o", bufs=2, space="PSUM") as psum_o,
            tc.tile_pool(name="dram", bufs=2, space="DRAM") as dram,
        ):
            # ---- load inputs (fine-grained DMAs spread across queues) ----
            # Issue order = queue assignment order: land the chunks the first
            # projection (Q, m=0) needs before everything else.
            xt_sb = persist.tile([128, 4, N], bf16)
            w_sb = persist.tile([128, 4, 3 * DIM], bf16)
            for kt in range(4):
                nc.sync.dma_start(xt_sb[:, kt, ts(0, 512)],
                                  xt[ts(kt, 128), ts(0, 512)])
            for kt in range(4):
                nc.sync.dma_start(w_sb[:, kt, ts(1, 512)],
                                  wqkv[ts(kt, 128), ts(1, 512)])
            for kt in range(4):
                nc.sync.dma_start(xt_sb[:, kt, ts(1, 512)],
                                  xt[ts(kt, 128), ts(1, 512)])
            for kt in range(4):
                nc.sync.dma_start(w_sb[:, kt, ts(0, 512)],
                                  wqkv[ts(kt, 128), ts(0, 512)])
            for cc in range(2, 4):
                for kt in range(4):
                    nc.sync.dma_start(xt_sb[:, kt, ts(cc, 512)],
                                      xt[ts(kt, 128), ts(cc, 512)])
            for kt in range(4):
                nc.sync.dma_start(w_sb[:, kt, ts(2, 512)],
                                  wqkv[ts(kt, 128), ts(2, 512)])
            wout_sb = persist.tile([128, 4, DIM], bf16)
            for kt in range(4):
                nc.sync.dma_start(wout_sb[:, kt, :], wout[ts(kt, 128), :])
            bout_sb = persist.tile([128, 4], f32)
            nc.sync.dma_start(bout_sb[:], bout[:, :])

            qt_sb = persist.tile([128, 4, NQ], bf16)
            kt_sb = persist.tile([128, 4, N], bf16)

            def q_proj(m, split_copy=False):
                # Q^T tile m: stationary = Wq tile, moving = X^T
                ps = psum_s.tile([128, NQ], mybir.dt.float32, tag="ps", name=f"psq{m}")
                for kt in range(4):
                    for c in range(2):
                        nc.tensor.matmul(
                            ps[:, ts(c, 512)],
                            lhsT=w_sb[:, kt, ts(m, 128)],
                            rhs=xt_sb[:, kt, ts(c, 512)],
                            start=(kt == 0), stop=(kt == 3),
                        )
                if split_copy:
                    # first chunk unblocks the first score matmuls sooner
                    nc.vector.tensor_copy(qt_sb[:, m, 0:512], ps[:, 0:512])
                    nc.vector.tensor_copy(qt_sb[:, m, 512:NQ], ps[:, 512:NQ])
                else:
                    nc.vector.tensor_copy(qt_sb[:, m, :], ps[:, :])

            def k_proj(m, cc, split_copy=False):
                ps = psum_s.tile([128, NQ], mybir.dt.float32, tag="ps",
                                 name=f"psk{m}_{cc}")
                for kt in range(4):
                    for c in range(2):
                        nc.tensor.matmul(
                            ps[:, ts(c, 512)],
                            lhsT=w_sb[:, kt, 512 + m * 128:512 + (m + 1) * 128],
                            rhs=xt_sb[:, kt, cc * 1024 + c * 512:cc * 1024 + (c + 1) * 512],
                            start=(kt == 0), stop=(kt == 3),
                        )
                if split_copy:
                    base = cc * 1024
                    nc.vector.tensor_copy(kt_sb[:, m, base:base + 256],
                                          ps[:, 0:256])
                    nc.vector.tensor_copy(kt_sb[:, m, base + 256:base + 1024],
                                          ps[:, 256:NQ])
                else:
                    nc.vector.tensor_copy(kt_sb[:, m, ts(cc, 1024)], ps[:, :])

            # split proj groups in halves: ~1us of PE work per injection slot
            # (the PSUM accumulation group stays open across the two halves)
            inj = {}

            def q_proj_part(m, half):
                if half == 0:
                    inj["q", m] = psum_s.tile([128, NQ], mybir.dt.float32,
                                              tag="ps", name=f"psq{m}")
                ps = inj["q", m]
                for kt in ((0, 1) if half == 0 else (2, 3)):
                    for c in range(2):
                        nc.tensor.matmul(
                            ps[:, ts(c, 512)],
                            lhsT=w_sb[:, kt, ts(m, 128)],
                            rhs=xt_sb[:, kt, ts(c, 512)],
                            start=(kt == 0), stop=(kt == 3),
                        )
                if half == 1:
                    nc.vector.tensor_copy(qt_sb[:, m, :], ps[:, :])

            def k_proj_part(m, cc, half):
                if half == 0:
                    inj["k", m, cc] = psum_s.tile([128, NQ], mybir.dt.float32,
                                                  tag="ps", name=f"psk{m}_{cc}")
                ps = inj["k", m, cc]
                for kt in ((0, 1) if half == 0 else (2, 3)):
                    for c in range(2):
                        nc.tensor.matmul(
                            ps[:, ts(c, 512)],
                            lhsT=w_sb[:, kt, 512 + m * 128:512 + (m + 1) * 128],
                            rhs=xt_sb[:, kt, cc * 1024 + c * 512:cc * 1024 + (c + 1) * 512],
                            start=(kt == 0), stop=(kt == 3),
                        )
                if half == 1:
                    nc.vector.tensor_copy(kt_sb[:, m, ts(cc, 1024)], ps[:, :])

            k_proj(0, 0, split_copy=True)
            q_proj(0, split_copy=True)
            k_proj(0, 1)

            # V natural [2048, 512] + ones column -> v_sb [128, nk_tile, head, 65]
            v_sb = persist.tile([128, 16, H, D + 1], bf16)
            nc.vector.memset(v_sb[:, :, :, D:D + 1], 1.0)

            def v_proj():
                for t in range(16):
                    ps = psum_o.tile([128, NQ], mybir.dt.float32, tag="po",
                                     name=f"psv{t}")
                    for kt in range(4):
                        nc.tensor.matmul(
                            ps[:, 0:512],
                            lhsT=xt_sb[:, kt, ts(t, 128)],
                            rhs=w_sb[:, kt, 1024:1536],
                            start=(kt == 0), stop=(kt == 3),
                        )
                    nc.vector.tensor_copy(
                        v_sb[:, t, :, 0:D],
                        ps[:, 0:512].rearrange("p (h d) -> p h d", h=H),
                    )

            # ---- attention, one head PAIR at a time ----
            # Head A lives on partitions 0:64, head B on 64:128 of K^T/Q^T
            # tile hp; their score matmuls target disjoint PE row groups
            # (tile_position row 0 vs 64), so adjacent ones run concurrently.
            ot_sb = persist.tile([128, 4, NQ], bf16)
            y_sb = persist.tile([128, 4, NQ], mybir.dt.float32)

            def y_fold_m(hp, m):
                # fold pair hp's slice of the output projection for e-tile m:
                # y_sb[m] += Wout[hp-block]^T @ O^T[hp-block]
                ps = psum_s.tile([128, NQ], mybir.dt.float32, tag="ps",
                                 name=f"psy{hp}_{m}")
                for c in range(2):
                    nc.tensor.matmul(
                        ps[:, ts(c, 512)],
                        lhsT=wout_sb[:, hp, ts(m, 128)],
                        rhs=ot_sb[:, hp, ts(c, 512)],
                        start=True, stop=True,
                    )
                if hp == 0:
                    nc.vector.tensor_copy(y_sb[:, m, :], ps[:, :])
                else:
                    nc.vector.tensor_add(y_sb[:, m, :], y_sb[:, m, :], ps[:, :])

            def y_fold(hp):
                for m in range(4):
                    y_fold_m(hp, m)

            for hp in range(4):
                hA, hB = 2 * hp, 2 * hp + 1
                # po accumulators allocated lazily (for pair 0 they must come
                # AFTER the V-projection's psum_o allocations)
                poA = poB = None
                # Software-pipelined: scores/exp for tile t are emitted one
                # iteration AHEAD of the PV matmuls for tile t-1, so freshly
                # unblocked score matmuls sit at the head of the PE FIFO
                # instead of behind the PV work (keeps ScalarE saturated).
                prev = None
                for t in range(17):
                    if t < 16:
                        ssA = psum_s.tile([128, NQ], mybir.dt.float32, tag="ps")
                        ssB = psum_s.tile([128, NQ], mybir.dt.float32, tag="ps")
                        for c in range(2):
                            nc.tensor.matmul(
                                ssA[:, ts(c, 512)],
                                lhsT=kt_sb[0:64, hp, ts(t, 128)],
                                rhs=qt_sb[0:64, hp, ts(c, 512)],
                                start=True, stop=True,
                            )
                        ptA = ptiles.tile([128, NQ], bf16, tag="pt")
                        nc.scalar.activation(ptA[:, :], ssA[:, :], Exp, scale=SCALE)
                    # V-projection rides here for pair 0: the first score
                    # group + exp are already emitted, so ScalarE ramps up
                    # while the PE grinds through the V matmuls.
                    if hp == 0 and t == 0:
                        v_proj()
                    if t >= 1:
                        if poA is None:
                            poA = psum_o.tile([128, NQ], mybir.dt.float32, tag="po")
                            poB = psum_o.tile([128, NQ], mybir.dt.float32, tag="po")
                        for c in range(2):
                            nc.tensor.matmul(
                                poA[0:D + 1, ts(c, 512)],
                                lhsT=v_sb[:, t - 1, hA, :],
                                rhs=prev[0][:, ts(c, 512)],
                                start=(t == 1), stop=(t == 16),
                            )
                    if t < 16:
                        for c in range(2):
                            nc.tensor.matmul(
                                ssB[:, ts(c, 512)],
                                lhsT=kt_sb[64:128, hp, ts(t, 128)],
                                rhs=qt_sb[64:128, hp, ts(c, 512)],
                                start=True, stop=True,
                            )
                        ptB = ptiles.tile([128, NQ], bf16, tag="pt")
                        nc.scalar.activation(ptB[:, :], ssB[:, :], Exp, scale=SCALE)
                    if t >= 1:
                        for c in range(2):
                            nc.tensor.matmul(
                                poB[0:D + 1, ts(c, 512)],
                                lhsT=v_sb[:, t - 1, hB, :],
                                rhs=prev[1][:, ts(c, 512)],
                                start=(t == 1), stop=(t == 16),
                            )
                    if t < 16:
                        prev = (ptA, ptB)
                    # ride independent work in the ACT-bound shadow, spread
                    # thin so no single iteration's PE budget blows up
                    if hp < 3:
                        if t == 4:
                            q_proj_part(hp + 1, 0)
                        elif t == 5:
                            q_proj_part(hp + 1, 1)
                        elif t == 7:
                            k_proj_part(hp + 1, 0, 0)
                        elif t == 8:
                            k_proj_part(hp + 1, 0, 1)
                        elif t == 10:
                            k_proj_part(hp + 1, 1, 0)
                        elif t == 11:
                            k_proj_part(hp + 1, 1, 1)
                    if hp in (1, 2) and 12 <= t <= 15:
                        y_fold_m(hp - 1, t - 12)
                    # pair 2's fold rides pair 3's early iterations so its
                    # VectorE adds stay off the tail critical path
                    if hp == 3 and 2 <= t <= 5:
                        y_fold_m(2, t - 2)
                # copy accumulators to SBUF so the PSUM banks free immediately
                # (on the last pair, put head B's copy on the now-idle ScalarE
                # so the two copies overlap)
                oas = []
                for po, eng in ((poA, "v"), (poB, "s" if hp == 3 else "v")):
                    oa = norm.tile([128, NQ], mybir.dt.float32, tag="oa")
                    if eng == "v":
                        nc.vector.tensor_copy(oa[0:D + 1, :], po[0:D + 1, :])
                    else:
                        nc.scalar.copy(oa[0:D + 1, :], po[0:D + 1, :])
                    oas.append(oa)
                # normalization, split per nq-chunk so the two chains overlap:
                # spread the sums across partitions for a wide reciprocal,
                # then broadcast via DRAM round-trip.
                for oa, hr in ((oas[0], 0), (oas[1], 1)):
                    sh = None
                    for c in range(2):
                        cs = ts(c, 512)
                        sp = norm.tile([128, 4], mybir.dt.float32, tag="sp", bufs=4)
                        nc.sync.dma_start(sp[:, :], oa[D:D + 1, cs])
                        rsp = norm.tile([128, 4], mybir.dt.float32, tag="rsp", bufs=4)
                        nc.vector.reciprocal(rsp[:, :], sp[:, :])
                        sd = dram.tile([1, 512], mybir.dt.float32, tag="sd", bufs=4)
                        nc.sync.dma_start(sd[:, :], rsp[:, :])
                        bc = norm.tile([128, 512], mybir.dt.float32, tag="bc", bufs=4)
                        bcast_ap = bass.AP(
                            tensor=sd.tensor, offset=sd.offset,
                            ap=[[0, 64], [1, 512]],
                        )
                        nc.sync.dma_start(bc[0:64, :], bcast_ap)
                        if hr == 0:
                            nc.vector.tensor_mul(ot_sb[0:64, hp, cs],
                                                 oa[0:64, cs], bc[0:64, :])
                        else:
                            if sh is None:
                                sh = norm.tile([128, NQ], bf16, tag="sh")
                            nc.vector.tensor_mul(sh[0:64, cs],
                                                 oa[0:64, cs], bc[0:64, :])
                            nc.sync.dma_start(ot_sb[64:128, hp, cs], sh[0:64, cs])
            # ---- fused tail: fold last pair + bias + store per e-tile,
            # chunked per half, contraction split per head so the head-A half
            # starts before head B's partition-shift DMA lands
            for m in range(4):
                ps = psum_s.tile([128, NQ], mybir.dt.float32, tag="ps",
                                 name=f"psyT{m}")
                ys = ysb.tile([128, NQ], mybir.dt.float32, tag="ys", bufs=4)
                for c in range(2):
                    cs = ts(c, 512)
                    nc.tensor.matmul(
                        ps[:, cs],
                        lhsT=wout_sb[:, 3, ts(m, 128)],
                        rhs=ot_sb[:, 3, cs],
                        start=True, stop=True,
                    )
                    nc.vector.scalar_tensor_tensor(
                        ys[:, cs], ps[:, cs], bout_sb[:, m:m + 1], y_sb[:, m, cs],
                        mybir.AluOpType.add, mybir.AluOpType.add,
                    )
                    nc.sync.dma_start(yt[ts(m, 128), cs], ys[:, cs])

    nc.compile()
    return nc


def _get_nc():
    if "nc" not in _CACHE:
        _CACHE["nc"] = _build()
    return _CACHE["nc"]


def kernel(x, w_qkv, w_out, b_out):
    global LAST_EXEC_TIME_NS
    from concourse.bass_utils import run_bass_kernel_spmd

    x = np.asarray(x, dtype=np.float32)
    w_qkv = np.asarray(w_qkv, dtype=np.float32)
    w_out = np.asarray(w_out, dtype=np.float32)
    b_out = np.asarray(b_out, dtype=np.float32)

    wqkv_b = w_qkv.astype(BF16)
    wout_b = w_out.astype(BF16)
    bout_t = np.ascontiguousarray(b_out.reshape(4, 128).T).astype(np.float32)

    in_maps = []
    for c in range(NCORES):
        b, qh = c // 2, c % 2
        q0 = qh * NQ
        xb = x[b]
        perm = np.concatenate([
            np.arange(q0, q0 + NQ),
            np.arange(0, q0),
            np.arange(q0 + NQ, N),
        ])
        xt = np.ascontiguousarray(xb[perm].T).astype(BF16)
        in_maps.append({
            "xt": xt,
            "wqkv": wqkv_b,
            "wout": wout_b,
            "bout": bout_t,
        })

    nc = _get_nc()
    trace = bool(int(os.environ.get("ATTN_TRACE", "0")))
    res = run_bass_kernel_spmd(nc, in_maps, core_ids=list(range(NCORES)),
                               trace=trace)
    LAST_EXEC_TIME_NS = res.exec_time_ns

    out = np.empty((B, N, DIM), np.float32)
    for c in range(NCORES):
        b, qh = c // 2, c % 2
        out[b, qh * NQ:(qh + 1) * NQ, :] = res.results[c]["out"].T
    return out


# revision 47
# speedup vs baseline: 1.0748x; 1.0748x over previous
"""Multi-head attention (B=4, N=2048, DIM=512, H=8) on 8 TRN2 NeuronCores.

Sharding: core c handles (batch = c//2, query-half = c%2) -> 1024 queries of
one batch, all heads. Zero collectives: K/V are recomputed per core pair
(keys are permuted so each core's queries come first; softmax is
permutation-invariant over keys).

Device layout ("transposed flash"):
  - everything dim-major: X^T, Q^T, K^T in SBUF with the contraction dim on
    partitions; V in natural [n, d] layout with a fused ones-column so the
    PV matmul also produces the softmax denominators (row 64 of the PSUM
    accumulator).
  - scores S^T = K_h^T-stationary @ Q_h^T-moving -> [nk_tile, nq] PSUM,
    exp on ScalarE (scale fused) -> P^T bf16 in SBUF,
    O^T_aug += V_aug^T-stationary @ P^T-moving accumulated over nk tiles.
  - normalization: reciprocal of the sums row, broadcast across partitions
    via a DRAM round-trip DMA (step-0 access pattern), multiply on VectorE.
  - output projection Wout-stationary gives Y^T [512, 1024]; host transposes.
"""

import os

import numpy as np
import ml_dtypes

B, N, DIM = 4, 2048, 512
H, D = 8, 64
NQ = 1024            # queries per core
NCORES = 8
SCALE = DIM ** -0.5  # reference scales by full dim, not head dim

BF16 = ml_dtypes.bfloat16

_CACHE = {}

LAST_EXEC_TIME_NS = None


def _build():
    import concourse.bass as bass
    import concourse.mybir as mybir
    import concourse.tile as tile
    from concourse import bacc

    f32 = mybir.dt.float32
    bf16 = mybir.dt.bfloat16
    Exp = mybir.ActivationFunctionType.Exp
    ts = bass.ts

    nc = bacc.Bacc("TRN2", target_bir_lowering=False, debug=False,
                   num_devices=NCORES)

    xt = nc.dram_tensor("xt", [DIM, N], bf16, kind="ExternalInput")
    wqkv = nc.dram_tensor("wqkv", [DIM, 3 * DIM], bf16, kind="ExternalInput")
    wout = nc.dram_tensor("wout", [DIM, DIM], bf16, kind="ExternalInput")
    bout = nc.dram_tensor("bout", [128, 4], f32, kind="ExternalInput")
    yt = nc.dram_tensor("out", [DIM, NQ], f32, kind="ExternalOutput")

    with tile.TileContext(nc) as tc:
        with (
            tc.tile_pool(name="persist", bufs=1) as persist,
            tc.tile_pool(name="ptiles", bufs=8) as ptiles,
            tc.tile_pool(name="norm", bufs=2) as norm,
            tc.tile_pool(name="ysb", bufs=2) as ysb,
            tc.tile_pool(name="psum_s", bufs=2, space="PSUM") as psum_s,
            tc.tile_pool(name="psum_o", bufs=2, space="PSUM") as psum_o,
            tc.tile_pool(name="dram", bufs=2, space="DRAM") as dram,
        ):
            # ---- load inputs (fine-grained DMAs spread across queues) ----
            # Issue order = queue assignment order: land the chunks the first
            # projection (Q, m=0) needs before everything else.
            xt_sb = persist.tile([128, 4, N], bf16)
            w_sb = persist.tile([128, 4, 3 * DIM], bf16)
            for kt in range(4):
                nc.sync.dma_start(xt_sb[:, kt, ts(0, 512)],
                                  xt[ts(kt, 128), ts(0, 512)])
            for kt in range(4):
                nc.sync.dma_start(w_sb[:, kt, ts(0, 512)],
                                  wqkv[ts(kt, 128), ts(0, 512)])
            for kt in range(4):
                nc.sync.dma_start(xt_sb[:, kt, ts(1, 512)],
                                  xt[ts(kt, 128), ts(1, 512)])
            for kt in range(4):
                nc.sync.dma_start(w_sb[:, kt, ts(1, 512)],
                                  wqkv[ts(kt, 128), ts(1, 512)])
            for cc in range(2, 4):
                for kt in range(4):
                    nc.sync.dma_start(xt_sb[:, kt, ts(cc, 512)],
                                      xt[ts(kt, 128), ts(cc, 512)])
            for kt in range(4):
                nc.sync.dma_start(w_sb[:, kt, ts(2, 512)],
                                  wqkv[ts(kt, 128), ts(2, 512)])
            wout_sb = persist.tile([128, 4, DIM], bf16)
            for kt in range(4):
                nc.sync.dma_start(wout_sb[:, kt, :], wout[ts(kt, 128), :])
            bout_sb = persist.tile([128, 4], f32)
            nc.sync.dma_start(bout_sb[:], bout[:, :])

            qt_sb = persist.tile([128, 4, NQ], bf16)
            kt_sb = persist.tile([128, 4, N], bf16)

            def q_proj(m, split_copy=False):
                # Q^T tile m: stationary = Wq tile, moving = X^T
                ps = psum_s.tile([128, NQ], mybir.dt.float32, tag="ps", name=f"psq{m}")
                for kt in range(4):
                    for c in range(2):
                        nc.tensor.matmul(
                            ps[:, ts(c, 512)],
                            lhsT=w_sb[:, kt, ts(m, 128)],
                            rhs=xt_sb[:, kt, ts(c, 512)],
                            start=(kt == 0), stop=(kt == 3),
                        )
                if split_copy:
                    # first chunk unblocks the first score matmuls sooner
                    nc.vector.tensor_copy(qt_sb[:, m, 0:512], ps[:, 0:512])
                    nc.vector.tensor_copy(qt_sb[:, m, 512:NQ], ps[:, 512:NQ])
                else:
                    nc.vector.tensor_copy(qt_sb[:, m, :], ps[:, :])

            def k_proj(m, cc, split_copy=False):
                ps = psum_s.tile([128, NQ], mybir.dt.float32, tag="ps",
                                 name=f"psk{m}_{cc}")
                for kt in range(4):
                    for c in range(2):
                        nc.tensor.matmul(
                            ps[:, ts(c, 512)],
                            lhsT=w_sb[:, kt, 512 + m * 128:512 + (m + 1) * 128],
                            rhs=xt_sb[:, kt, cc * 1024 + c * 512:cc * 1024 + (c + 1) * 512],
                            start=(kt == 0), stop=(kt == 3),
                        )
                if split_copy:
                    base = cc * 1024
                    nc.vector.tensor_copy(kt_sb[:, m, base:base + 256],
                                          ps[:, 0:256])
                    nc.vector.tensor_copy(kt_sb[:, m, base + 256:base + 1024],
                                          ps[:, 256:NQ])
                else:
                    nc.vector.tensor_copy(kt_sb[:, m, ts(cc, 1024)], ps[:, :])

            # split proj groups in halves: ~1us of PE work per injection slot
            # (the PSUM accumulation group stays open across the two halves)
            inj = {}

            def q_proj_part(m, half):
                if half == 0:
                    inj["q", m] = psum_s.tile([128, NQ], mybir.dt.float32,
                                              tag="ps", name=f"psq{m}")
                ps = inj["q", m]
                for kt in ((0, 1) if half == 0 else (2, 3)):
                    for c in range(2):
                        nc.tensor.matmul(
                            ps[:, ts(c, 512)],
                            lhsT=w_sb[:, kt, ts(m, 128)],
                            rhs=xt_sb[:, kt, ts(c, 512)],
                            start=(kt == 0), stop=(kt == 3),
                        )
                if half == 1:
                    nc.vector.tensor_copy(qt_sb[:, m, :], ps[:, :])

            def k_proj_part(m, cc, half):
                if half == 0:
                    inj["k", m, cc] = psum_s.tile([128, NQ], mybir.dt.float32,
                                                  tag="ps", name=f"psk{m}_{cc}")
                ps = inj["k", m, cc]
                for kt in ((0, 1) if half == 0 else (2, 3)):
                    for c in range(2):
                        nc.tensor.matmul(
                            ps[:, ts(c, 512)],
                            lhsT=w_sb[:, kt, 512 + m * 128:512 + (m + 1) * 128],
                            rhs=xt_sb[:, kt, cc * 1024 + c * 512:cc * 1024 + (c + 1) * 512],
                            start=(kt == 0), stop=(kt == 3),
                        )
                if half == 1:
                    nc.vector.tensor_copy(kt_sb[:, m, ts(cc, 1024)], ps[:, :])

            k_proj(0, 0, split_copy=True)
            q_proj(0, split_copy=True)
            k_proj(0, 1)

            # V natural [2048, 512] + ones column -> v_sb [128, nk_tile, head, 65]
            v_sb = persist.tile([128, 16, H, D + 1], bf16)
            nc.vector.memset(v_sb[:, :, :, D:D + 1], 1.0)

            def v_proj():
                for t in range(16):
                    ps = psum_o.tile([128, NQ], mybir.dt.float32, tag="po",
                                     name=f"psv{t}")
                    for kt in range(4):
                        nc.tensor.matmul(
                            ps[:, 0:512],
                            lhsT=xt_sb[:, kt, ts(t, 128)],
                            rhs=w_sb[:, kt, 1024:1536],
                            start=(kt == 0), stop=(kt == 3),
                        )
                    nc.vector.tensor_copy(
                        v_sb[:, t, :, 0:D],
                        ps[:, 0:512].rearrange("p (h d) -> p h d", h=H),
                    )

            # ---- attention, one head PAIR at a time ----
            # Head A lives on partitions 0:64, head B on 64:128 of K^T/Q^T
            # tile hp; their score matmuls target disjoint PE row groups
            # (tile_position row 0 vs 64), so adjacent ones run concurrently.
            ot_sb = persist.tile([128, 4, NQ], bf16)
            y_sb = persist.tile([128, 4, NQ], mybir.dt.float32)

            def y_fold_m(hp, m):
                # fold pair hp's slice of the output projection for e-tile m:
                # y_sb[m] += Wout[hp-block]^T @ O^T[hp-block]
                ps = psum_s.tile([128, NQ], mybir.dt.float32, tag="ps",
                                 name=f"psy{hp}_{m}")
                for c in range(2):
                    nc.tensor.matmul(
                        ps[:, ts(c, 512)],
                        lhsT=wout_sb[:, hp, ts(m, 128)],
                        rhs=ot_sb[:, hp, ts(c, 512)],
                        start=True, stop=True,
                    )
                if hp == 0:
                    nc.vector.tensor_copy(y_sb[:, m, :], ps[:, :])
                else:
                    nc.vector.tensor_add(y_sb[:, m, :], y_sb[:, m, :], ps[:, :])

            def y_fold(hp):
                for m in range(4):
                    y_fold_m(hp, m)

            for hp in range(4):
                hA, hB = 2 * hp, 2 * hp + 1
                # po accumulators allocated lazily (for pair 0 they must come
                # AFTER the V-projection's psum_o allocations)
                poA = poB = None
                # Software-pipelined: scores/exp for tile t are emitted one
                # iteration AHEAD of the PV matmuls for tile t-1, so freshly
                # unblocked score matmuls sit at the head of the PE FIFO
                # instead of behind the PV work (keeps ScalarE saturated).
                prev = None
                for t in range(17):
                    if t < 16:
                        ssA = psum_s.tile([128, NQ], mybir.dt.float32, tag="ps")
                        ssB = psum_s.tile([128, NQ], mybir.dt.float32, tag="ps")
                        for c in range(2):
                            nc.tensor.matmul(
                                ssA[:, ts(c, 512)],
                                lhsT=kt_sb[0:64, hp, ts(t, 128)],
                                rhs=qt_sb[0:64, hp, ts(c, 512)],
                                start=True, stop=True,
                            )
                        ptA = ptiles.tile([128, NQ], bf16, tag="pt")
                        nc.scalar.activation(ptA[:, :], ssA[:, :], Exp, scale=SCALE)
                    # V-projection rides here for pair 0: the first score
                    # group + exp are already emitted, so ScalarE ramps up
                    # while the PE grinds through the V matmuls.
                    if hp == 0 and t == 0:
                        v_proj()
                    if t >= 1:
                        if poA is None:
                            poA = psum_o.tile([128, NQ], mybir.dt.float32, tag="po")
                            poB = psum_o.tile([128, NQ], mybir.dt.float32, tag="po")
                        for c in range(2):
                            nc.tensor.matmul(
                                poA[0:D + 1, ts(c, 512)],
                                lhsT=v_sb[:, t - 1, hA, :],
                                rhs=prev[0][:, ts(c, 512)],
                                start=(t == 1), stop=(t == 16),
                            )
                    if t < 16:
                        for c in range(2):
                            nc.tensor.matmul(
                                ssB[:, ts(c, 512)],
                                lhsT=kt_sb[64:128, hp, ts(t, 128)],
                                rhs=qt_sb[64:128, hp, ts(c, 512)],
                                start=True, stop=True,
                            )
                        ptB = ptiles.tile([128, NQ], bf16, tag="pt")
                        nc.scalar.activation(ptB[:, :], ssB[:, :], Exp, scale=SCALE)
                    if t >= 1:
                        for c in range(2):
                            nc.tensor.matmul(
                                poB[0:D + 1, ts(c, 512)],
                                lhsT=v_sb[:, t - 1, hB, :],
                                rhs=prev[1][:, ts(c, 512)],
                                start=(t == 1), stop=(t == 16),
                            )
                    if t < 16:
                        prev = (ptA, ptB)
                    # ride independent work in the ACT-bound shadow, spread
                    # thin so no single iteration's PE budget blows up
                    if hp < 3:
                        if t == 4:
                            q_proj_part(hp + 1, 0)
                        elif t == 5:
                            q_proj_part(hp + 1, 1)
                        elif t == 7:
                            k_proj_part(hp + 1, 0, 0)
                        elif t == 8:
                            k_proj_part(hp + 1, 0, 1)
                        elif t == 10:
                            k_proj_part(hp + 1, 1, 0)
                        elif t == 11:
                            k_proj_part(hp + 1, 1, 1)
                    if hp in (1, 2) and 12 <= t <= 15:
                        y_fold_m(hp - 1, t - 12)
                    # pair 2's fold rides pair 3's late iterations, landing
                    # between PV emissions so it fills the PE's wait windows
                    if hp == 3 and 13 <= t <= 16:
                        y_fold_m(2, t - 13)
                # copy accumulators to SBUF so the PSUM banks free immediately
                # (on the last pair, put head B's copy on the now-idle ScalarE
                # so the two copies overlap)
                oas = []
                for po, eng in ((poA, "v"), (poB, "s" if hp == 3 else "v")):
                    oa = norm.tile([128, NQ], mybir.dt.float32, tag="oa")
                    if eng == "v":
                        nc.vector.tensor_copy(oa[0:D + 1, :], po[0:D + 1, :])
                    else:
                        nc.scalar.copy(oa[0:D + 1, :], po[0:D + 1, :])
                    oas.append(oa)
                # normalization, split per nq-chunk so the two chains overlap:
                # spread the sums across partitions for a wide reciprocal,
                # then broadcast via DRAM round-trip.
                for oa, hr in ((oas[0], 0), (oas[1], 1)):
                    sh = None
                    for c in range(2):
                        cs = ts(c, 512)
                        sp = norm.tile([128, 4], mybir.dt.float32, tag="sp", bufs=4)
                        nc.sync.dma_start(sp[:, :], oa[D:D + 1, cs])
                        rsp = norm.tile([128, 4], mybir.dt.float32, tag="rsp", bufs=4)
                        nc.vector.reciprocal(rsp[:, :], sp[:, :])
                        sd = dram.tile([1, 512], mybir.dt.float32, tag="sd", bufs=4)
                        nc.sync.dma_start(sd[:, :], rsp[:, :])
                        bc = norm.tile([128, 512], mybir.dt.float32, tag="bc", bufs=4)
                        bcast_ap = bass.AP(
                            tensor=sd.tensor, offset=sd.offset,
                            ap=[[0, 64], [1, 512]],
                        )
                        nc.sync.dma_start(bc[0:64, :], bcast_ap)
                        if hr == 0:
                            nc.vector.tensor_mul(ot_sb[0:64, hp, cs],
                                                 oa[0:64, cs], bc[0:64, :])
                        else:
                            if sh is None:
                                sh = norm.tile([128, NQ], bf16, tag="sh")
                            nc.vector.tensor_mul(sh[0:64, cs],
                                                 oa[0:64, cs], bc[0:64, :])
                            nc.sync.dma_start(ot_sb[64:128, hp, cs], sh[0:64, cs])
            # ---- fused tail: fold last pair + bias + store per e-tile,
            # chunked per half, contraction split per head so the head-A half
            # starts before head B's partition-shift DMA lands
            for m in range(4):
                ps = psum_s.tile([128, NQ], mybir.dt.float32, tag="ps",
                                 name=f"psyT{m}")
                ys = ysb.tile([128, NQ], mybir.dt.float32, tag="ys", bufs=4)
                for c in range(2):
                    cs = ts(c, 512)
                    nc.tensor.matmul(
                        ps[:, cs],
                        lhsT=wout_sb[:, 3, ts(m, 128)],
                        rhs=ot_sb[:, 3, cs],
                        start=True, stop=True,
                    )
                    nc.vector.scalar_tensor_tensor(
                        ys[:, cs], ps[:, cs], bout_sb[:, m:m + 1], y_sb[:, m, cs],
                        mybir.AluOpType.add, mybir.AluOpType.add,
                    )
                    nc.sync.dma_start(yt[ts(m, 128), cs], ys[:, cs])

    nc.compile()
    return nc


def _get_nc():
    if "nc" not in _CACHE:
        _CACHE["nc"] = _build()
    return _CACHE["nc"]


def kernel(x, w_qkv, w_out, b_out):
    global LAST_EXEC_TIME_NS
    from concourse.bass_utils import run_bass_kernel_spmd

    x = np.asarray(x, dtype=np.float32)
    w_qkv = np.asarray(w_qkv, dtype=np.float32)
    w_out = np.asarray(w_out, dtype=np.float32)
    b_out = np.asarray(b_out, dtype=np.float32)

    wqkv_b = w_qkv.astype(BF16)
    wout_b = w_out.astype(BF16)
    bout_t = np.ascontiguousarray(b_out.reshape(4, 128).T).astype(np.float32)

    in_maps = []
    for c in range(NCORES):
        b, qh = c // 2, c % 2
        q0 = qh * NQ
        xb = x[b]
        perm = np.concatenate([
            np.arange(q0, q0 + NQ),
            np.arange(0, q0),
            np.arange(q0 + NQ, N),
        ])
        xt = np.ascontiguousarray(xb[perm].T).astype(BF16)
        in_maps.append({
            "xt": xt,
            "wqkv": wqkv_b,
            "wout": wout_b,
            "bout": bout_t,
        })

    nc = _get_nc()
    trace = bool(int(os.environ.get("ATTN_TRACE", "0")))
    res = run_bass_kernel_spmd(nc, in_maps, core_ids=list(range(NCORES)),
                               trace=trace)
    LAST_EXEC_TIME_NS = res.exec_time_ns

    out = np.empty((B, N, DIM), np.float32)
    for c in range(NCORES):
        b, qh = c // 2, c % 2
        out[b, qh * NQ:(qh + 1) * NQ, :] = res.results[c]["out"].T
    return out


# revision 48
# speedup vs baseline: 1.0797x; 1.0045x over previous
"""Multi-head attention (B=4, N=2048, DIM=512, H=8) on 8 TRN2 NeuronCores.

Sharding: core c handles (batch = c//2, query-half = c%2) -> 1024 queries of
one batch, all heads. Zero collectives: K/V are recomputed per core pair
(keys are permuted so each core's queries come first; softmax is
permutation-invariant over keys).

Device layout ("transposed flash"):
  - everything dim-major: X^T, Q^T, K^T in SBUF with the contraction dim on
    partitions; V in natural [n, d] layout with a fused ones-column so the
    PV matmul also produces the softmax denominators (row 64 of the PSUM
    accumulator).
  - scores S^T = K_h^T-stationary @ Q_h^T-moving -> [nk_tile, nq] PSUM,
    exp on ScalarE (scale fused) -> P^T bf16 in SBUF,
    O^T_aug += V_aug^T-stationary @ P^T-moving accumulated over nk tiles.
  - normalization: reciprocal of the sums row, broadcast across partitions
    via a DRAM round-trip DMA (step-0 access pattern), multiply on VectorE.
  - output projection Wout-stationary gives Y^T [512, 1024]; host transposes.
"""

import os

import numpy as np
import ml_dtypes

B, N, DIM = 4, 2048, 512
H, D = 8, 64
NQ = 1024            # queries per core
NCORES = 8
SCALE = DIM ** -0.5  # reference scales by full dim, not head dim

BF16 = ml_dtypes.bfloat16

_CACHE = {}

LAST_EXEC_TIME_NS = None


def _build():
    import concourse.bass as bass
    import concourse.mybir as mybir
    import concourse.tile as tile
    from concourse import bacc

    f32 = mybir.dt.float32
    bf16 = mybir.dt.bfloat16
    Exp = mybir.ActivationFunctionType.Exp
    ts = bass.ts

    nc = bacc.Bacc("TRN2", target_bir_lowering=False, debug=False,
                   num_devices=NCORES)

    xt = nc.dram_tensor("xt", [DIM, N], bf16, kind="ExternalInput")
    wqkv = nc.dram_tensor("wqkv", [DIM, 3 * DIM], bf16, kind="ExternalInput")
    wout = nc.dram_tensor("wout", [DIM, DIM], bf16, kind="ExternalInput")
    bout = nc.dram_tensor("bout", [128, 4], f32, kind="ExternalInput")
    yt = nc.dram_tensor("out", [DIM, NQ], f32, kind="ExternalOutput")

    with tile.TileContext(nc) as tc:
        with (
            tc.tile_pool(name="persist", bufs=1) as persist,
            tc.tile_pool(name="ptiles", bufs=8) as ptiles,
            tc.tile_pool(name="norm", bufs=2) as norm,
            tc.tile_pool(name="ysb", bufs=2) as ysb,
            tc.tile_pool(name="psum_s", bufs=2, space="PSUM") as psum_s,
            tc.tile_pool(name="psum_o", bufs=2, space="PSUM") as psum_o,
            tc.tile_pool(name="dram", bufs=2, space="DRAM") as dram,
        ):
            # ---- load inputs (fine-grained DMAs spread across queues) ----
            # Issue order = queue assignment order: land the chunks the first
            # projection (Q, m=0) needs before everything else.
            xt_sb = persist.tile([128, 4, N], bf16)
            w_sb = persist.tile([128, 4, 3 * DIM], bf16)
            for kt in range(4):
                nc.sync.dma_start(xt_sb[:, kt, ts(0, 512)],
                                  xt[ts(kt, 128), ts(0, 512)])
            for kt in range(4):
                nc.sync.dma_start(w_sb[:, kt, ts(1, 512)],
                                  wqkv[ts(kt, 128), ts(1, 512)])
            for kt in range(4):
                nc.sync.dma_start(xt_sb[:, kt, ts(1, 512)],
                                  xt[ts(kt, 128), ts(1, 512)])
            for kt in range(4):
                nc.sync.dma_start(w_sb[:, kt, ts(0, 512)],
                                  wqkv[ts(kt, 128), ts(0, 512)])
            for cc in range(2, 4):
                for kt in range(4):
                    nc.sync.dma_start(xt_sb[:, kt, ts(cc, 512)],
                                      xt[ts(kt, 128), ts(cc, 512)])
            for kt in range(4):
                nc.sync.dma_start(w_sb[:, kt, ts(2, 512)],
                                  wqkv[ts(kt, 128), ts(2, 512)])
            wout_sb = persist.tile([128, 4, DIM], bf16)
            for kt in range(4):
                nc.sync.dma_start(wout_sb[:, kt, :], wout[ts(kt, 128), :])
            bout_sb = persist.tile([128, 4], f32)
            nc.sync.dma_start(bout_sb[:], bout[:, :])

            qt_sb = persist.tile([128, 4, NQ], bf16)
            kt_sb = persist.tile([128, 4, N], bf16)

            def q_proj(m, split_copy=False):
                # Q^T tile m: stationary = Wq tile, moving = X^T
                ps = psum_s.tile([128, NQ], mybir.dt.float32, tag="ps", name=f"psq{m}")
                for kt in range(4):
                    for c in range(2):
                        nc.tensor.matmul(
                            ps[:, ts(c, 512)],
                            lhsT=w_sb[:, kt, ts(m, 128)],
                            rhs=xt_sb[:, kt, ts(c, 512)],
                            start=(kt == 0), stop=(kt == 3),
                        )
                if split_copy:
                    # first chunk unblocks the first score matmuls sooner
                    nc.vector.tensor_copy(qt_sb[:, m, 0:512], ps[:, 0:512])
                    nc.vector.tensor_copy(qt_sb[:, m, 512:NQ], ps[:, 512:NQ])
                else:
                    nc.vector.tensor_copy(qt_sb[:, m, :], ps[:, :])

            def k_proj(m, cc, split_copy=False):
                ps = psum_s.tile([128, NQ], mybir.dt.float32, tag="ps",
                                 name=f"psk{m}_{cc}")
                for kt in range(4):
                    for c in range(2):
                        nc.tensor.matmul(
                            ps[:, ts(c, 512)],
                            lhsT=w_sb[:, kt, 512 + m * 128:512 + (m + 1) * 128],
                            rhs=xt_sb[:, kt, cc * 1024 + c * 512:cc * 1024 + (c + 1) * 512],
                            start=(kt == 0), stop=(kt == 3),
                        )
                if split_copy:
                    base = cc * 1024
                    nc.vector.tensor_copy(kt_sb[:, m, base:base + 256],
                                          ps[:, 0:256])
                    nc.vector.tensor_copy(kt_sb[:, m, base + 256:base + 1024],
                                          ps[:, 256:NQ])
                else:
                    nc.vector.tensor_copy(kt_sb[:, m, ts(cc, 1024)], ps[:, :])

            # split proj groups in halves: ~1us of PE work per injection slot
            # (the PSUM accumulation group stays open across the two halves)
            inj = {}

            def q_proj_part(m, half):
                if half == 0:
                    inj["q", m] = psum_s.tile([128, NQ], mybir.dt.float32,
                                              tag="ps", name=f"psq{m}")
                ps = inj["q", m]
                for kt in ((0, 1) if half == 0 else (2, 3)):
                    for c in range(2):
                        nc.tensor.matmul(
                            ps[:, ts(c, 512)],
                            lhsT=w_sb[:, kt, ts(m, 128)],
                            rhs=xt_sb[:, kt, ts(c, 512)],
                            start=(kt == 0), stop=(kt == 3),
                        )
                if half == 1:
                    nc.vector.tensor_copy(qt_sb[:, m, :], ps[:, :])

            def k_proj_part(m, cc, half):
                if half == 0:
                    inj["k", m, cc] = psum_s.tile([128, NQ], mybir.dt.float32,
                                                  tag="ps", name=f"psk{m}_{cc}")
                ps = inj["k", m, cc]
                for kt in ((0, 1) if half == 0 else (2, 3)):
                    for c in range(2):
                        nc.tensor.matmul(
                            ps[:, ts(c, 512)],
                            lhsT=w_sb[:, kt, 512 + m * 128:512 + (m + 1) * 128],
                            rhs=xt_sb[:, kt, cc * 1024 + c * 512:cc * 1024 + (c + 1) * 512],
                            start=(kt == 0), stop=(kt == 3),
                        )
                if half == 1:
                    nc.vector.tensor_copy(kt_sb[:, m, ts(cc, 1024)], ps[:, :])

            k_proj(0, 0, split_copy=True)
            q_proj(0, split_copy=True)
            k_proj(0, 1)

            # V natural [2048, 512] + ones column -> v_sb [128, nk_tile, head, 65]
            v_sb = persist.tile([128, 16, H, D + 1], bf16)
            nc.vector.memset(v_sb[:, :, :, D:D + 1], 1.0)

            def v_proj():
                for t in range(16):
                    ps = psum_o.tile([128, NQ], mybir.dt.float32, tag="po",
                                     name=f"psv{t}")
                    for kt in range(4):
                        nc.tensor.matmul(
                            ps[:, 0:512],
                            lhsT=xt_sb[:, kt, ts(t, 128)],
                            rhs=w_sb[:, kt, 1024:1536],
                            start=(kt == 0), stop=(kt == 3),
                        )
                    nc.vector.tensor_copy(
                        v_sb[:, t, :, 0:D],
                        ps[:, 0:512].rearrange("p (h d) -> p h d", h=H),
                    )

            # ---- attention, one head PAIR at a time ----
            # Head A lives on partitions 0:64, head B on 64:128 of K^T/Q^T
            # tile hp; their score matmuls target disjoint PE row groups
            # (tile_position row 0 vs 64), so adjacent ones run concurrently.
            ot_sb = persist.tile([128, 4, NQ], bf16)
            y_sb = persist.tile([128, 4, NQ], mybir.dt.float32)

            def y_fold_m(hp, m):
                # fold pair hp's slice of the output projection for e-tile m:
                # y_sb[m] += Wout[hp-block]^T @ O^T[hp-block]
                ps = psum_s.tile([128, NQ], mybir.dt.float32, tag="ps",
                                 name=f"psy{hp}_{m}")
                for c in range(2):
                    nc.tensor.matmul(
                        ps[:, ts(c, 512)],
                        lhsT=wout_sb[:, hp, ts(m, 128)],
                        rhs=ot_sb[:, hp, ts(c, 512)],
                        start=True, stop=True,
                    )
                if hp == 0:
                    nc.vector.tensor_copy(y_sb[:, m, :], ps[:, :])
                else:
                    nc.vector.tensor_add(y_sb[:, m, :], y_sb[:, m, :], ps[:, :])

            def y_fold(hp):
                for m in range(4):
                    y_fold_m(hp, m)

            for hp in range(4):
                hA, hB = 2 * hp, 2 * hp + 1
                # po accumulators allocated lazily (for pair 0 they must come
                # AFTER the V-projection's psum_o allocations)
                poA = poB = None
                # Software-pipelined: scores/exp for tile t are emitted one
                # iteration AHEAD of the PV matmuls for tile t-1, so freshly
                # unblocked score matmuls sit at the head of the PE FIFO
                # instead of behind the PV work (keeps ScalarE saturated).
                prev = None
                for t in range(17):
                    if t < 16:
                        ssA = psum_s.tile([128, NQ], mybir.dt.float32, tag="ps")
                        ssB = psum_s.tile([128, NQ], mybir.dt.float32, tag="ps")
                        for c in range(2):
                            nc.tensor.matmul(
                                ssA[:, ts(c, 512)],
                                lhsT=kt_sb[0:64, hp, ts(t, 128)],
                                rhs=qt_sb[0:64, hp, ts(c, 512)],
                                start=True, stop=True,
                            )
                        ptA = ptiles.tile([128, NQ], bf16, tag="pt")
                        nc.scalar.activation(ptA[:, :], ssA[:, :], Exp, scale=SCALE)
                    # V-projection rides here for pair 0: the first score
                    # group + exp are already emitted, so ScalarE ramps up
                    # while the PE grinds through the V matmuls.
                    if hp == 0 and t == 0:
                        v_proj()
                    if t >= 1:
                        if poA is None:
                            poA = psum_o.tile([128, NQ], mybir.dt.float32, tag="po")
                            poB = psum_o.tile([128, NQ], mybir.dt.float32, tag="po")
                        for c in range(2):
                            nc.tensor.matmul(
                                poA[0:D + 1, ts(c, 512)],
                                lhsT=v_sb[:, t - 1, hA, :],
                                rhs=prev[0][:, ts(c, 512)],
                                start=(t == 1), stop=(t == 16),
                            )
                    if t < 16:
                        for c in range(2):
                            nc.tensor.matmul(
                                ssB[:, ts(c, 512)],
                                lhsT=kt_sb[64:128, hp, ts(t, 128)],
                                rhs=qt_sb[64:128, hp, ts(c, 512)],
                                start=True, stop=True,
                            )
                        ptB = ptiles.tile([128, NQ], bf16, tag="pt")
                        nc.scalar.activation(ptB[:, :], ssB[:, :], Exp, scale=SCALE)
                    if t >= 1:
                        for c in range(2):
                            nc.tensor.matmul(
                                poB[0:D + 1, ts(c, 512)],
                                lhsT=v_sb[:, t - 1, hB, :],
                                rhs=prev[1][:, ts(c, 512)],
                                start=(t == 1), stop=(t == 16),
                            )
                    if t < 16:
                        prev = (ptA, ptB)
                    # ride independent work in the ACT-bound shadow, spread
                    # thin so no single iteration's PE budget blows up
                    if hp < 3:
                        if t == 4:
                            q_proj_part(hp + 1, 0)
                        elif t == 5:
                            q_proj_part(hp + 1, 1)
                        elif t == 7:
                            k_proj_part(hp + 1, 0, 0)
                        elif t == 8:
                            k_proj_part(hp + 1, 0, 1)
                        elif t == 10:
                            k_proj_part(hp + 1, 1, 0)
                        elif t == 11:
                            k_proj_part(hp + 1, 1, 1)
                    if hp in (1, 2) and 12 <= t <= 15:
                        y_fold_m(hp - 1, t - 12)
                    # pair 2's fold rides pair 3's late iterations, landing
                    # between PV emissions so it fills the PE's wait windows
                    if hp == 3 and 13 <= t <= 16:
                        y_fold_m(2, t - 13)
                # copy accumulators to SBUF so the PSUM banks free immediately
                # (on the last pair, put head B's copy on the now-idle ScalarE
                # so the two copies overlap)
                oas = []
                for po, eng in ((poA, "v"), (poB, "s" if hp == 3 else "v")):
                    oa = norm.tile([128, NQ], mybir.dt.float32, tag="oa")
                    if eng == "v":
                        nc.vector.tensor_copy(oa[0:D + 1, :], po[0:D + 1, :])
                    else:
                        nc.scalar.copy(oa[0:D + 1, :], po[0:D + 1, :])
                    oas.append(oa)
                # normalization, split per nq-chunk so the two chains overlap:
                # spread the sums across partitions for a wide reciprocal,
                # then broadcast via DRAM round-trip.
                for oa, hr in ((oas[0], 0), (oas[1], 1)):
                    sh = None
                    for c in range(2):
                        cs = ts(c, 512)
                        sp = norm.tile([128, 4], mybir.dt.float32, tag="sp", bufs=4)
                        nc.sync.dma_start(sp[:, :], oa[D:D + 1, cs])
                        rsp = norm.tile([128, 4], mybir.dt.float32, tag="rsp", bufs=4)
                        nc.vector.reciprocal(rsp[:, :], sp[:, :])
                        sd = dram.tile([1, 512], mybir.dt.float32, tag="sd", bufs=4)
                        nc.sync.dma_start(sd[:, :], rsp[:, :])
                        bc = norm.tile([128, 512], mybir.dt.float32, tag="bc", bufs=4)
                        bcast_ap = bass.AP(
                            tensor=sd.tensor, offset=sd.offset,
                            ap=[[0, 64], [1, 512]],
                        )
                        nc.sync.dma_start(bc[0:64, :], bcast_ap)
                        if hr == 0:
                            nc.vector.tensor_mul(ot_sb[0:64, hp, cs],
                                                 oa[0:64, cs], bc[0:64, :])
                        else:
                            if sh is None:
                                sh = norm.tile([128, NQ], bf16, tag="sh")
                            nc.vector.tensor_mul(sh[0:64, cs],
                                                 oa[0:64, cs], bc[0:64, :])
                            nc.sync.dma_start(ot_sb[64:128, hp, cs], sh[0:64, cs])
            # ---- fused tail: fold last pair + bias + store per e-tile,
            # chunked per half, contraction split per head so the head-A half
            # starts before head B's partition-shift DMA lands
            for m in range(4):
                ps = psum_s.tile([128, NQ], mybir.dt.float32, tag="ps",
                                 name=f"psyT{m}")
                ys = ysb.tile([128, NQ], mybir.dt.float32, tag="ys", bufs=4)
                for c in range(2):
                    cs = ts(c, 512)
                    nc.tensor.matmul(
                        ps[:, cs],
                        lhsT=wout_sb[:, 3, ts(m, 128)],
                        rhs=ot_sb[:, 3, cs],
                        start=True, stop=True,
                    )
                    nc.vector.scalar_tensor_tensor(
                        ys[:, cs], ps[:, cs], bout_sb[:, m:m + 1], y_sb[:, m, cs],
                        mybir.AluOpType.add, mybir.AluOpType.add,
                    )
                    nc.sync.dma_start(yt[ts(m, 128), cs], ys[:, cs])

    nc.compile()
    return nc


def _get_nc():
    if "nc" not in _CACHE:
        _CACHE["nc"] = _build()
    return _CACHE["nc"]


def kernel(x, w_qkv, w_out, b_out):
    global LAST_EXEC_TIME_NS
    from concourse.bass_utils import run_bass_kernel_spmd

    x = np.asarray(x, dtype=np.float32)
    w_qkv = np.asarray(w_qkv, dtype=np.float32)
    w_out = np.asarray(w_out, dtype=np.float32)
    b_out = np.asarray(b_out, dtype=np.float32)

    wqkv_b = w_qkv.astype(BF16)
    wout_b = w_out.astype(BF16)
    bout_t = np.ascontiguousarray(b_out.reshape(4, 128).T).astype(np.float32)

    in_maps = []
    for c in range(NCORES):
        b, qh = c // 2, c % 2
        q0 = qh * NQ
        xb = x[b]
        perm = np.concatenate([
            np.arange(q0, q0 + NQ),
            np.arange(0, q0),
            np.arange(q0 + NQ, N),
        ])
        xt = np.ascontiguousarray(xb[perm].T).astype(BF16)
        in_maps.append({
            "xt": xt,
            "wqkv": wqkv_b,
            "wout": wout_b,
            "bout": bout_t,
        })

    nc = _get_nc()
    trace = bool(int(os.environ.get("ATTN_TRACE", "0")))
    res = run_bass_kernel_spmd(nc, in_maps, core_ids=list(range(NCORES)),
                               trace=trace)
    LAST_EXEC_TIME_NS = res.exec_time_ns

    out = np.empty((B, N, DIM), np.float32)
    for c in range(NCORES):
        b, qh = c // 2, c % 2
        out[b, qh * NQ:(qh + 1) * NQ, :] = res.results[c]["out"].T
    return out
